# revision 35
# baseline (speedup 1.0000x reference)
"""Multi-head self-attention (RoPE + causal) Trainium2 Bass kernel.

Problem: b=2, s=2048, d_model=1024, 16 heads x 64 dims, causal, RoPE.
Sharding over 8 NeuronCores: core c -> (batch bi = c//4, head group g = c%4
of 4 heads). Each core computes its 4 heads' attention from x[bi] and
produces a partial output projection (Wo column-block); the host sums the
4 partials per batch element.

v10 layout (all matmul operands bf16, fp32 PSUM accumulate):
  inputs  8 early large DMAs (stripe-pair tiles) + 2 deferred (x half-1,
          WoT) on the sync HWDGE ring; each dma_start costs ~0.6us of
          issue time on its engine and ring backpressure paces issues to
          HBM, so DMA count is minimized and rope-swap DMAs are emitted
          so they outrank the bulk loads in queue order
  QK proj lhsT = W slices, psum [128,1024] (two 4-kc matmul steps so the
          ring slot is held briefly); RoPE = evict copy + SB->SB DMA
          32-row swap + 3 aligned DVE muls
  scores  ST[k,q] per (pair t, kblock j, qchunk): both heads in one
          [128, 2, 512] psum ring (3 deep) via base-partition subarray
          concurrency (K=64 row tiles), exp on ACT (scale 1/8) emitted 2
          chunks ahead of AV; causal mask = one DVE mul on the diagonal
  AV      per-head per-qchunk single-bank accumulators [128, 512]; V row
          64 = ones column so the softmax denominator rides along
  norm    batched heads: av evicts split ACT||DVE (~0.7us release; the
          next window first AV is a PE-FIFO head-of-line block on it),
          one recip [1,1024], one gpsimd partition_broadcast, 2 muls
  sched   pair-major attention windows (A0* then A1*) with the later
          projections emitted between windows for the Tile scheduler to
          interleave; output projection is H-major: H=0 (queries 0-1023)
          runs during the final normalize chain keeping the PE HAM-warm,
          H=1 follows; per (dm,H) evict (alt ACT/DVE) + 256KB DMA
"""

import os
import sys
from contextlib import ExitStack

import numpy as np

for _p in ("/root/.axon_site", "/root/.axon_site/_ro/trn_rl_repo", "/opt/trn_rl_repo"):
    if os.path.isdir(_p) and _p not in sys.path:
        sys.path.append(_p)

import ml_dtypes  # noqa: E402
import concourse.bass as bass  # noqa: E402
import concourse.tile as tile  # noqa: E402
import concourse.mybir as mybir  # noqa: E402
from concourse import bacc  # noqa: E402
from concourse.bass import ts  # noqa: E402
from concourse.bass_utils import run_bass_kernel_spmd  # noqa: E402

BF16 = mybir.dt.bfloat16
F32 = mybir.dt.float32
NPBF16 = ml_dtypes.bfloat16

S = 2048
D = 1024
DK = 64
THETA = 10000.0

_CACHE = {}


def _build_nc():
    nc = bacc.Bacc("TRN2", target_bir_lowering=False, debug=False, num_devices=8)
    xT = nc.dram_tensor("xT", [D, S], BF16, kind="ExternalInput").ap()
    wqkv = nc.dram_tensor("wqkv", [D, 768], BF16, kind="ExternalInput").ap()
    woT = nc.dram_tensor("woT", [256, D], BF16, kind="ExternalInput").ap()
    ropec = nc.dram_tensor("ropec", [128, 2048], BF16, kind="ExternalInput").ap()
    ropes = nc.dram_tensor("ropes", [128, 2048], BF16, kind="ExternalInput").ap()
    yp = nc.dram_tensor("yp", [D, S], BF16, kind="ExternalOutput").ap()

    Exp = mybir.ActivationFunctionType.Exp

    with ExitStack() as ctx:
        tc = ctx.enter_context(tile.TileContext(nc))
        const = ctx.enter_context(tc.tile_pool(name="const", bufs=1))
        sb = ctx.enter_context(tc.tile_pool(name="sb", bufs=3))
        expp = ctx.enter_context(tc.tile_pool(name="expp", bufs=8))
        outp = ctx.enter_context(tc.tile_pool(name="outp", bufs=3))
        ps = ctx.enter_context(tc.tile_pool(name="ps", bufs=3, space="PSUM"))
        psav = ctx.enter_context(tc.tile_pool(name="psav", bufs=1, space="PSUM"))

        # ---- persistent SBUF ----
        # single tiles holding all 128-row stripes so each input loads with
        # ONE large DMA: a HWDGE dma_start occupies the issuing engine's
        # queue ~0.6us and ring backpressure paces issues to HBM, so 26
        # small loads would hog whichever engine queue issues them.
        # x half-0 and the weights are split into four physical tiles of 2
        # stripes each: Tile tracks DMA deps per tile, so each projection
        # matmul releases as soon as its own ~0.9MB lands (DMA-paced
        # pipeline through the whole contraction).
        xts_h0 = [const.tile([128, 2, 1024], BF16, tag=f"x0{a}", name=f"x0{a}")
                  for a in range(4)]
        xts_h1 = const.tile([128, 8, 1024], BF16, tag="x1", name="x1")
        wts_sp = [const.tile([128, 2, 768], BF16, tag=f"wt{a}", name=f"wt{a}")
                  for a in range(4)]
        wos_all = const.tile([128, 2, 1024], BF16, tag="woa", name="woa")

        def xts(h, kc):
            if h == 1:
                return xts_h1[:, kc, :]
            return xts_h0[kc // 2][:, kc % 2, :]

        def wts(kc):
            return wts_sp[kc // 2][:, kc % 2, :]
        ropec_sb = const.tile([128, 2048], BF16, tag="ropec")
        ropes_sb = const.tile([128, 2048], BF16, tag="ropes")
        # V per key-block: 4 heads x (64 dims + ones col)
        vt = [const.tile([128, 260], BF16, tag=f"v{j}", name=f"v{j}") for j in range(16)]
        # Q/K projected+roped halves: [t][half] -> [128, 1024]
        qf = [[const.tile([128, 1024], BF16, tag=f"qf{t}_{h}", name=f"qf{t}_{h}")
               for h in range(2)] for t in range(2)]
        kf = [[const.tile([128, 1024], BF16, tag=f"kf{t}_{h}", name=f"kf{t}_{h}")
               for h in range(2)] for t in range(2)]
        # attention output O^T per pair: rows = 2 heads x 64 dims
        ot = [const.tile([128, S], BF16, tag=f"ot{t}", name=f"ot{t}") for t in range(2)]

        # ---- input DMAs: 6 early + 2 deferred large transfers ----
        wqkvr = wqkv.rearrange("(i p) c -> p i c", p=128)
        xTr = xT.rearrange("(i p) c -> p i c", p=128)
        woTr = woT.rearrange("(i p) c -> p i c", p=128)
        for a in range(2):
            nc.sync.dma_start(wts_sp[a][:], wqkvr[:, 2 * a : 2 * a + 2, :])
            nc.sync.dma_start(xts_h0[a][:], xTr[:, 2 * a : 2 * a + 2, 0:1024])
        nc.sync.dma_start(ropec_sb[:], ropec[:])
        nc.sync.dma_start(ropes_sb[:], ropes[:])
        for a in range(2, 4):
            nc.sync.dma_start(wts_sp[a][:], wqkvr[:, 2 * a : 2 * a + 2, :])
            nc.sync.dma_start(xts_h0[a][:], xTr[:, 2 * a : 2 * a + 2, 0:1024])

        def load_late_inputs():
            # emitted after the first rope swaps so those jump this queue
            nc.sync.dma_start(xts_h1[:], xTr[:, :, 1024:2048])
            nc.sync.dma_start(wos_all[:], woTr[:])

        # ones columns of vt tiles
        for j in range(16):
            ones_ap = vt[j][:].rearrange("p (h x) -> p h x", h=4)[:, :, 64:65]
            nc.gpsimd.memset(ones_ap, 1.0)

        # causal triangle mask tile, doubled for both heads of a pair:
        # tri2[k, hh, c] = 1 if c >= k else 0
        tri2 = const.tile([128, 2, 128], BF16, tag="tri2")
        nc.gpsimd.memset(tri2[:], 1.0)
        nc.gpsimd.affine_select(
            out=tri2[:], in_=tri2[:], compare_op=mybir.AluOpType.is_ge,
            fill=0.0, base=0, pattern=[[0, 2], [1, 128]], channel_multiplier=-1,
        )

        # ---- V projection: V[key, vdim] natural layout ----
        def v_step(st, inter=False):
            def f():
                vp = ps.tile([128, 256], F32, tag="big", name="vp")
                for kc in range(8):
                    nc.tensor.matmul(
                        vp[:],
                        lhsT=xts(st // 8, kc)[:, ts(st % 8, 128)],
                        rhs=wts(kc)[:, 512:768],
                        start=(kc == 0),
                        stop=(kc == 7),
                    )
                dst = vt[st][:].rearrange("p (h x) -> p h x", h=4)[:, :, 0:64]
                vsrc = vp[:].rearrange("p (h x) -> p h x", h=4)
                if inter:
                    nc.vector.tensor_copy(dst, vsrc)
                else:
                    nc.scalar.copy(dst, vsrc)
            return f

        def vproj(lo, hi, inter=False):
            for st in range(lo, hi):
                v_step(st, inter)()

        # ---- Q/K projection + RoPE (as a list of feeder steps) ----
        def qk_steps(t, qk, H, act_evict=False):
            wcol = (0 if qk == 0 else 256) + t * 128
            dstt = qf[t] if qk == 0 else kf[t]
            state = {}

            def mm_step(kc0, kn):
                def f():
                    if kc0 == 0:
                        state["qkp"] = ps.tile(
                            [128, 1024], F32, tag="big", name="qkp"
                        )
                    qkp = state["qkp"]
                    for kc in range(kc0, kc0 + kn):
                        for c in range(2):
                            nc.tensor.matmul(
                                qkp[:, ts(c, 512)],
                                lhsT=wts(kc)[:, wcol : wcol + 128],
                                rhs=xts(H, kc)[:, ts(c, 512)],
                                start=(kc == 0),
                                stop=(kc == 7),
                            )
                return f

            def rope_tail():
                qkp = state["qkp"]
                kb = sb.tile([128, 1024], BF16, tag=f"kb{qk}", name="kb")
                if act_evict:
                    nc.scalar.copy(kb[:], qkp[:])
                else:
                    nc.vector.tensor_copy(kb[:], qkp[:])
                kbs = sb.tile([128, 1024], BF16, tag=f"kbs{qk}", name="kbs")
                nc.sync.dma_start(kbs[0:32, :], kb[32:64, :])
                nc.sync.dma_start(kbs[32:64, :], kb[0:32, :])
                nc.sync.dma_start(kbs[64:96, :], kb[96:128, :])
                nc.sync.dma_start(kbs[96:128, :], kb[64:96, :])
                t1 = sb.tile([128, 1024], BF16, tag=f"t1{qk}", name="t1")
                nc.vector.tensor_mul(t1[:], kb[:], ropec_sb[:, ts(H, 1024)])
                t2 = sb.tile([128, 1024], BF16, tag=f"t2{qk}", name="t2")
                nc.vector.tensor_mul(t2[:], kbs[:], ropes_sb[:, ts(H, 1024)])
                nc.vector.tensor_add(dstt[H][:], t1[:], t2[:])

            # two big matmul steps (not four) so the PSUM slot is held across
            # fewer feeder slots — long holds back up the shared PSUM ring
            # and stall independent work behind the rope chain
            return [mm_step(0, 4), mm_step(4, 4), rope_tail]

        def qk_proj(t, qk, H, act_evict=False):
            for f in qk_steps(t, qk, H, act_evict):
                f()

        # ---- attention for pair t, single qchunk qc (512 queries) ----
        # Software-pipelined: scores for chunk j+LOOKAHEAD are emitted before
        # the AV matmuls of chunk j, so the PE never waits for ACT's exp.
        def attn(t, qc, feeders=(), per_chunk=2):
            fq = list(feeders)

            def feed(n):
                for _ in range(n):
                    if fq:
                        f = fq.pop(0)
                        if f is not None:
                            f()

            av = [
                psav.tile([128, 512], F32, tag=f"av{hh}", name=f"av{hh}")
                for hh in range(2)
            ]
            js = list(range(4 * qc + 4))
            stage = {}

            def scores(j):
                q0 = 128 * j
                c_start = max(512 * qc, q0)
                w = 512 * (qc + 1) - c_start
                H = qc // 2
                cl = c_start - 1024 * H
                sps = ps.tile([128, 2, 512], F32, tag="big", name="sps")
                es = expp.tile([128, 2, 512], BF16, tag="es", name="es")
                for hh in range(2):
                    r0 = 64 * hh
                    nc.tensor.matmul(
                        sps[:, hh, 0:w],
                        lhsT=kf[t][j // 8][r0 : r0 + 64, ts(j % 8, 128)],
                        rhs=qf[t][H][r0 : r0 + 64, cl : cl + w],
                        start=True,
                        stop=True,
                    )
                nc.scalar.activation(es[:, :, 0:w], sps[:, :, 0:w], Exp, scale=0.125)
                if c_start == q0:
                    nc.vector.tensor_mul(
                        es[:, :, 0:128], es[:, :, 0:128], tri2[:]
                    )
                stage[j] = (es, c_start % 512, w)

            def accum(j):
                es, lo, w = stage.pop(j)
                for hh in range(2):
                    hv = 2 * t + hh
                    nc.tensor.matmul(
                        av[hh][0:65, lo : lo + w],
                        lhsT=vt[j][:, 65 * hv : 65 * hv + 65],
                        rhs=es[:, hh, 0:w],
                        start=(j == 0),
                        stop=(j == js[-1]),
                    )

            LOOK = 2
            for i, j in enumerate(js):
                scores(j)
                if i >= LOOK:
                    accum(js[i - LOOK])
                feed(per_chunk)
            for j in js[-LOOK:]:
                accum(j)
            feed(len(fq))

            # normalize, both heads batched. The evicts release the PSUM
            # accumulators, which gates the NEXT window's first AV matmul
            # (a PE-FIFO head-of-line block), so head 0 evicts on ACT and
            # head 1 on DVE concurrently (~0.7us release).
            oasb = sb.tile([64, 1024], BF16, tag="oasb", name="oasb")
            dn = sb.tile([1, 1024], F32, tag="dn", name="dn")
            nc.scalar.copy(oasb[:, 0:512], av[0][0:64, :])
            nc.scalar.copy(dn[:, 0:512], av[0][64:65, :])
            nc.vector.tensor_copy(oasb[:, 512:1024], av[1][0:64, :])
            nc.vector.tensor_copy(dn[:, 512:1024], av[1][64:65, :])
            rr = sb.tile([1, 1024], F32, tag="rr", name="rr")
            nc.vector.reciprocal_approx_fast(rr[:], dn[:])
            rb = sb.tile([64, 1024], F32, tag="rb", name="rb")
            nc.gpsimd.partition_broadcast(rb[:], rr[:])
            for hh in range(2):
                nc.vector.tensor_mul(
                    ot[t][64 * hh : 64 * hh + 64, ts(qc, 512)],
                    oasb[:, ts(hh, 512)],
                    rb[:, ts(hh, 512)],
                )
            return []

        # ---- output projection, H-major ----
        # H=0 (queries 0-1023) depends only on attn qc 0-1 of both pairs, so
        # its 16 matmuls run during the final attention's normalize chain
        # (keeping the PE HAM-warm); H=1 follows. Each (dm, H) half evicts
        # and DMAs immediately. Few large DMAs: each dma_start costs ~0.6us
        # of serialized issue time on the sync engine.
        def outproj(H):
            for dm in range(8):
                pp = ps.tile([128, 1024], F32, tag="big", name="pp")
                for cc in range(2):
                    for c in range(2):
                        nc.tensor.matmul(
                            pp[:, ts(c, 512)],
                            lhsT=wos_all[:, cc, ts(dm, 128)],
                            rhs=ot[cc][:, 1024 * H + 512 * c : 1024 * H + 512 * (c + 1)],
                            start=(cc == 0),
                            stop=(cc == 1),
                        )
                yt = outp.tile([128, 1024], BF16, tag="yt", name="yt")
                if dm % 2 == 0:
                    nc.scalar.copy(yt[:], pp[:])
                else:
                    nc.vector.tensor_copy(yt[:], pp[:])
                nc.sync.dma_start(yp[ts(dm, 128), ts(H, 1024)], yt[:])

        # ---- emission schedule (qc-major) ----
        # Minimal pre-attention phase: K/Q for pair 0 half 0 + V blocks 0-3.
        # Everything else (later projections, V blocks, per-qc output
        # projection) feeds one step at a time into the PE slack under the
        # ACT-bound exp stream.
        # ---- emission schedule (baseline pair-major structure) ----
        qk_proj(0, 1, 0, act_evict=True)  # K pair 0 half 0
        qk_proj(0, 0, 0, act_evict=True)  # Q pair 0 half 0
        load_late_inputs()
        vproj(0, 8)
        vproj(8, 16)
        qk_proj(0, 0, 1)  # Q pair 0 half 1 first: A02 needs qf[0][1] at
        qk_proj(0, 1, 1)  # chunk 0, kf[0][1] only from chunk 9
        attn(0, 0)
        attn(0, 1)
        qk_proj(1, 1, 0)  # K pair 1 half 0 (rope during A02/A03)
        qk_proj(1, 0, 0)
        attn(0, 2)
        attn(0, 3)
        qk_proj(1, 0, 1)  # Q pair 1 half 1 first: A12 needs qf[1][1] at
        qk_proj(1, 1, 1)  # chunk 0, kf[1][1] only from chunk 9
        attn(1, 0)
        attn(1, 1)
        attn(1, 2)
        attn(1, 3)
        outproj(0)
        outproj(1)

    nc.compile()
    return nc


def _host_inputs(x, token_positions, Wq, Wk, Wv, Wo):
    x = np.asarray(x, dtype=np.float32)
    Wq = np.asarray(Wq, dtype=np.float32)
    Wk = np.asarray(Wk, dtype=np.float32)
    Wv = np.asarray(Wv, dtype=np.float32)
    Wo = np.asarray(Wo, dtype=np.float32)
    pos = np.asarray(token_positions).astype(np.float32)

    # rope tables, rows = [x1(32) x2(32)] x2 heads, freq index p%32
    f = np.arange(32, dtype=np.float32)
    inv = 1.0 / (THETA ** (2.0 * f / DK))
    ang = pos[:, None] * inv[None, :]  # [S, 32]
    cosT = np.cos(ang).T.astype(np.float32)  # [32, S]
    sinT = np.sin(ang).T.astype(np.float32)
    crow = np.tile(cosT, (4, 1))
    srow = np.concatenate([-sinT, sinT, -sinT, sinT], axis=0)

    ropec = np.ascontiguousarray(crow).astype(NPBF16)
    ropes = np.ascontiguousarray(srow).astype(NPBF16)

    ev = np.arange(0, DK, 2)
    od = np.arange(1, DK, 2)
    in_maps = []
    for core in range(8):
        bi, g = core // 4, core % 4
        xTb = np.ascontiguousarray(x[bi].T).astype(NPBF16)
        qk_idx = []
        for t in range(2):
            for hh, sel in ((2 * t, ev), (2 * t, od), (2 * t + 1, ev), (2 * t + 1, od)):
                qk_idx.append(DK * (4 * g + hh) + sel)
        qk_idx = np.concatenate(qk_idx)
        v_idx = 256 * g + np.arange(256)
        wq = Wq[qk_idx, :].T
        wk = Wk[qk_idx, :].T
        wv = Wv[v_idx, :].T
        wqkv = np.ascontiguousarray(
            np.concatenate([wq, wk, wv], axis=1)
        ).astype(NPBF16)
        woTl = np.ascontiguousarray(Wo[:, v_idx].T).astype(NPBF16)
        in_maps.append(
            dict(xT=xTb, wqkv=wqkv, woT=woTl, ropec=ropec, ropes=ropes)
        )
    return in_maps


def _run(inputs, trace=False, tmpdir=None):
    if "nc" not in _CACHE:
        _CACHE["nc"] = _build_nc()
    nc = _CACHE["nc"]
    in_maps = _host_inputs(**inputs)
    kw = {}
    if trace:
        kw = dict(trace=True, tmpdir=tmpdir)
    res = run_bass_kernel_spmd(nc, in_maps, list(range(8)), **kw)
    out = np.zeros((2, S, D), np.float32)
    for core in range(8):
        out[core // 4] += res.results[core]["yp"].astype(np.float32).T
    return out, res


def kernel(**inputs):
    out, _ = _run(inputs, trace=False)
    return out
